# revision 5
# baseline (speedup 1.0000x reference)
"""DeltaNet chunked delta-rule kernel for Trainium2 (Bass/Tile), 8-core SPMD.

Full inputs: q,k,v [4,8,4096,128] fp32, beta [4,8,4096] fp32.
Sharding: 32 (b,h) pairs -> 4 per core across 8 cores (state S is per (b,h)).

Algorithm (mathematically identical to the CHUNK=32 reference for any chunk
size; we use C=128):
  qh = l2norm(q), kh = l2norm(k), vb = v*beta, kb = kh*beta
  per chunk:  T = kb @ kh^T;  M = I + tril(T,-1);  inv = M^-1
              (computed exactly via the nilpotent Neumann product
               inv = prod_j (I + Mp^(2^j)), Mp = -tril(T,-1))
              u0 = inv @ vb ; w = inv @ kb ; attn = tril(qh kh^T)
  scan:       u = u0 - w @ S ; out = qh @ S + attn @ u ; S += kh^T u
"""
import numpy as np

import concourse.bass as bass
import concourse.mybir as mybir
import concourse.tile as tile
from concourse import bacc
from concourse.bass_utils import run_bass_kernel_spmd
from concourse.masks import make_identity, make_lower_triangular, make_upper_triangular

B, H, L, D = 4, 8, 4096, 128
C = 128
NT = L // C
NSEQ = (B * H) // 8   # sequences per core
FP = mybir.dt.float32
EPS = 1e-6
AF = mybir.ActivationFunctionType
ALU = mybir.AluOpType
AX = mybir.AxisListType


def _emit_tile(nc, work, pp, pscan, dram, S, bT, s, t):
    q_d, k_d, v_d, o_d = dram["q"], dram["k"], dram["v"], dram["out"]
    ident, mSL, mSU, mUI, epsT = dram["ident"], dram["mSL"], dram["mSU"], dram["mUI"], dram["epsT"]
    rows = slice(t * C, (t + 1) * C)

    # ---- loads ----
    qt = work.tile([C, D], FP, tag="q", name="qt")
    kt = work.tile([C, D], FP, tag="k", name="kt")
    vt = work.tile([C, D], FP, tag="v", name="vt")
    nc.sync.dma_start(out=qt, in_=q_d[s, rows, :])
    nc.sync.dma_start(out=kt, in_=k_d[s, rows, :])
    nc.sync.dma_start(out=vt, in_=v_d[s, rows, :])
    beta_col = bT[s][:, t:t + 1]  # [128,1] per-token beta

    # ---- l2 norms + beta scaling ----
    scr = work.tile([C, D], FP, tag="scr", name="scr")
    scr2 = work.tile([C, D], FP, tag="scr2", name="scr2")
    qss = work.tile([C, 1], FP, tag="qss", name="qss")
    kss = work.tile([C, 1], FP, tag="kss", name="kss")
    nc.vector.tensor_mul(scr, qt, qt)
    nc.vector.tensor_reduce(out=qss, in_=scr, axis=AX.X, op=ALU.add)
    nc.vector.tensor_mul(scr2, kt, kt)
    nc.vector.tensor_reduce(out=kss, in_=scr2, axis=AX.X, op=ALU.add)
    # sqrt(ss + eps) then reciprocal
    nc.scalar.activation(out=qss, in_=qss, func=AF.Sqrt, bias=epsT[:, 0:1], scale=1.0)
    nc.scalar.activation(out=kss, in_=kss, func=AF.Sqrt, bias=epsT[:, 0:1], scale=1.0)
    nc.vector.reciprocal(out=qss, in_=qss)
    nc.vector.reciprocal(out=kss, in_=kss)
    # qh = q * qr (in place), kh = k * kr (in place), vb = v*beta (in place),
    # kb = k * (kr*beta) via two-scalar tensor_scalar
    kb = work.tile([C, D], FP, tag="kb", name="kb")
    nc.gpsimd.tensor_scalar(out=kb, in0=kt, scalar1=kss[:, 0:1], scalar2=beta_col,
                            op0=ALU.mult, op1=ALU.mult)
    nc.gpsimd.tensor_scalar_mul(qt, qt, qss[:, 0:1])
    nc.gpsimd.tensor_scalar_mul(kt, kt, kss[:, 0:1])
    nc.gpsimd.tensor_scalar_mul(vt, vt, beta_col)

    # ---- transposes (d-major copies) ----
    qT_ps = pp.tile([D, C], FP, tag="pa", name="qT_ps")
    nc.tensor.transpose(qT_ps, qt, ident)
    qT = work.tile([D, C], FP, tag="qT", name="qT")
    nc.scalar.copy(qT, qT_ps)
    kT_ps = pp.tile([D, C], FP, tag="pa", name="kT_ps")
    nc.tensor.transpose(kT_ps, kt, ident)
    kT = work.tile([D, C], FP, tag="kT", name="kT")
    nc.scalar.copy(kT, kT_ps)
    kbT_ps = pp.tile([D, C], FP, tag="pa", name="kbT_ps")
    nc.tensor.transpose(kbT_ps, kb, ident)
    kbTn = work.tile([D, C], FP, tag="kbTn", name="kbTn")
    nc.scalar.activation(out=kbTn, in_=kbT_ps, func=AF.Copy, scale=-1.0)  # -kb^T

    # ---- T (negated) and seeds ----
    # T_ps = -T = -(kb kh^T);  Tt_ps = -T^T
    T_ps = pp.tile([C, C], FP, tag="pa", name="T_ps")
    nc.tensor.matmul(T_ps, kbTn, kT, start=True, stop=True)
    Tt_ps = pp.tile([C, C], FP, tag="pa", name="Tt_ps")
    nc.tensor.matmul(Tt_ps, kT, kbTn, start=True, stop=True)
    # P0 = -tril(T,-1) = tril(-T,-1);  PT0 = P0^T
    P = work.tile([C, C], FP, tag="Pb0", name="P0")
    nc.vector.tensor_mul(P, T_ps, mSL)
    PT = work.tile([C, C], FP, tag="PTb0", name="PT0")
    nc.vector.tensor_mul(PT, Tt_ps, mSU)
    F = work.tile([C, C], FP, tag="F0", name="F0")
    nc.vector.tensor_add(F, P, ident)       # F_j = I + P_j  (factor lhsT)
    R = work.tile([C, C], FP, tag="R", name="R0")
    nc.vector.tensor_add(R, PT, ident)      # running product: invT = prod (I + PT_j)

    # ---- Neumann chain: 7 factors (exponents 1..64) ----
    for j in range(6):
        # P_{j+1} = P_j^2 = mm(lhsT=PT_j, rhs=P_j)
        Pn_ps = pp.tile([C, C], FP, tag="pa", name="Pn_ps")
        nc.tensor.matmul(Pn_ps, PT, P, start=True, stop=True)
        Fn = work.tile([C, C], FP, tag=f"F{j+1}", name=f"F{j+1}")
        nc.vector.tensor_add(Fn, Pn_ps, ident)
        if j < 5:
            Pn = work.tile([C, C], FP, tag=f"Pb{j+1}", name=f"Pb{j+1}")
            nc.scalar.copy(Pn, Pn_ps)
            # PT_{j+1} = mm(lhsT=P_j, rhs=PT_j)
            PTn_ps = pp.tile([C, C], FP, tag="pa", name="PTn_ps")
            nc.tensor.matmul(PTn_ps, P, PT, start=True, stop=True)
            PTn = work.tile([C, C], FP, tag=f"PTb{j+1}", name=f"PTb{j+1}")
            nc.scalar.copy(PTn, PTn_ps)
            P, PT = Pn, PTn
        # R = (I + PT_{j+1}) R = mm(lhsT=F_{j+1}, rhs=R)
        Rn_ps = pp.tile([C, C], FP, tag="pa", name="Rn_ps")
        nc.tensor.matmul(Rn_ps, Fn, R, start=True, stop=True)
        R = work.tile([C, C], FP, tag="R", name=f"R{j+1}")
        nc.vector.tensor_copy(R, Rn_ps)
    invT = R  # [e, c] = inv^T

    # ---- u0, w^T (negated), attn^T ----
    u0_ps = pp.tile([C, D], FP, tag="pa", name="u0_ps")
    nc.tensor.matmul(u0_ps, invT, vt, start=True, stop=True)      # inv @ vb
    u0 = work.tile([C, D], FP, tag="u0", name="u0")
    nc.scalar.copy(u0, u0_ps)
    w_ps = pp.tile([D, C], FP, tag="pa", name="w_ps")
    nc.tensor.matmul(w_ps, kb, invT, start=True, stop=True)       # w^T = kb^T invT
    wTn = work.tile([D, C], FP, tag="wTn", name="wTn")
    nc.scalar.activation(out=wTn, in_=w_ps, func=AF.Copy, scale=-1.0)
    a_ps = pp.tile([C, C], FP, tag="pa", name="a_ps")
    nc.tensor.matmul(a_ps, kT, qT, start=True, stop=True)         # (qh kh^T)^T
    attnT = work.tile([C, C], FP, tag="attnT", name="attnT")
    nc.vector.tensor_mul(attnT, a_ps, mUI)                        # keep c2<=c1

    # ---- scan step ----
    St = S[s]
    u_ps = pscan.tile([C, D], FP, tag="u", name="u_ps")
    nc.tensor.matmul(u_ps, wTn, St, start=True, stop=True)        # -w @ S
    u = work.tile([C, D], FP, tag="u", name="u")
    nc.vector.tensor_add(u, u_ps, u0)                             # u = u0 - w S
    out_ps = pscan.tile([C, D], FP, tag="out", name="out_ps")
    nc.tensor.matmul(out_ps, qT, St, start=True, stop=False)      # qh @ S
    nc.tensor.matmul(out_ps, attnT, u, start=False, stop=True)    # += attn @ u
    sd_ps = pscan.tile([D, D], FP, tag="sd", name="sd_ps")
    nc.tensor.matmul(sd_ps, kt, u, start=True, stop=True)         # kh^T u
    nc.vector.tensor_add(St, St, sd_ps)                           # S += kh^T u
    out_sb = work.tile([C, D], FP, tag="outsb", name="out_sb")
    nc.scalar.copy(out_sb, out_ps)
    nc.sync.dma_start(out=o_d[s, rows, :], in_=out_sb)


def build_nc(nseq=NSEQ, nt=NT):
    ll = nt * C
    nc = bacc.Bacc(None, target_bir_lowering=False)
    dram = {
        "q": nc.dram_tensor("q", [nseq, ll, D], FP, kind="ExternalInput"),
        "k": nc.dram_tensor("k", [nseq, ll, D], FP, kind="ExternalInput"),
        "v": nc.dram_tensor("v", [nseq, ll, D], FP, kind="ExternalInput"),
        "beta": nc.dram_tensor("beta", [nseq, ll], FP, kind="ExternalInput"),
        "out": nc.dram_tensor("out", [nseq, ll, D], FP, kind="ExternalOutput"),
    }
    with tile.TileContext(nc) as tc:
        with (
            tc.tile_pool(name="consts", bufs=1) as consts,
            tc.tile_pool(name="persist", bufs=1) as persist,
            tc.tile_pool(name="work", bufs=2) as work,
            tc.tile_pool(name="pp", bufs=2, space="PSUM") as pp,
            tc.tile_pool(name="pscan", bufs=2, space="PSUM") as pscan,
        ):
            ident = consts.tile([128, 128], FP, tag="ident", name="ident")
            mSL = consts.tile([128, 128], FP, tag="mSL", name="mSL")
            mSU = consts.tile([128, 128], FP, tag="mSU", name="mSU")
            mUI = consts.tile([128, 128], FP, tag="mUI", name="mUI")
            epsT = consts.tile([128, 1], FP, tag="epsT", name="epsT")
            make_identity(nc, ident)
            make_lower_triangular(nc, mSL, val=1.0, diag=False)
            make_upper_triangular(nc, mSU, val=1.0, diag=False)
            make_upper_triangular(nc, mUI, val=1.0, diag=True)
            nc.gpsimd.memset(epsT, EPS)
            dram.update(ident=ident, mSL=mSL, mSU=mSU, mUI=mUI, epsT=epsT)

            S, bT = [], []
            for s in range(nseq):
                St = persist.tile([D, D], FP, tag=f"S{s}", name=f"S{s}")
                nc.gpsimd.memset(St, 0.0)
                S.append(St)
                bseq = persist.tile([nt, C], FP, tag=f"bseq{s}", name=f"bseq{s}")
                nc.sync.dma_start(out=bseq, in_=dram["beta"][s].rearrange("(n c) -> n c", c=C))
                bt_ps = pp.tile([C, nt], FP, tag="pa", name=f"btps{s}")
                nc.tensor.transpose(bt_ps, bseq, ident[:nt, :nt])
                btile = persist.tile([C, nt], FP, tag=f"bT{s}", name=f"bT{s}")
                nc.vector.tensor_copy(btile, bt_ps)
                bT.append(btile)

            for t in range(nt):
                for s in range(nseq):
                    _emit_tile(nc, work, pp, pscan, dram, S, bT, s, t)
    nc.compile()
    return nc


_NC_CACHE = None


def _build_in_maps(inputs):
    q = np.ascontiguousarray(np.asarray(inputs["q"], dtype=np.float32))
    k = np.ascontiguousarray(np.asarray(inputs["k"], dtype=np.float32))
    v = np.ascontiguousarray(np.asarray(inputs["v"], dtype=np.float32))
    beta = np.ascontiguousarray(np.asarray(inputs["beta"], dtype=np.float32))
    qf = q.reshape(B * H, L, D)
    kf = k.reshape(B * H, L, D)
    vf = v.reshape(B * H, L, D)
    bf = beta.reshape(B * H, L)
    in_maps = []
    for core in range(8):
        sl = slice(core * NSEQ, (core + 1) * NSEQ)
        in_maps.append({
            "q": np.ascontiguousarray(qf[sl]),
            "k": np.ascontiguousarray(kf[sl]),
            "v": np.ascontiguousarray(vf[sl]),
            "beta": np.ascontiguousarray(bf[sl]),
        })
    return in_maps


def kernel(q, k, v, beta):
    global _NC_CACHE
    if _NC_CACHE is None:
        _NC_CACHE = build_nc()
    nc = _NC_CACHE
    in_maps = _build_in_maps({"q": q, "k": k, "v": v, "beta": beta})
    res = run_bass_kernel_spmd(nc, in_maps, core_ids=list(range(8)))
    out = np.empty((B * H, L, D), dtype=np.float32)
    for core in range(8):
        out[core * NSEQ:(core + 1) * NSEQ] = res.results[core]["out"]
    return out.reshape(B, H, L, D)


# revision 14
# speedup vs baseline: 73.3497x; 73.3497x over previous
"""DeltaNet chunked delta-rule kernel for Trainium2 (Bass/Tile), 8-core SPMD.

Full inputs: q,k,v [4,8,4096,128] fp32, beta [4,8,4096] fp32.
Sharding: 32 (b,h) pairs -> 4 per core across 8 cores (state S is per (b,h)).

Algorithm (mathematically identical to the CHUNK=32 reference for any chunk
size; we use C=128):
  qh = l2norm(q), kh = l2norm(k), vb = v*beta, kb = kh*beta
  per chunk:  T = kb @ kh^T;  M = I + tril(T,-1);  inv = M^-1
              (computed exactly via the nilpotent Neumann product
               inv = prod_j (I + Mp^(2^j)), Mp = -tril(T,-1))
              u0 = inv @ vb ; w = inv @ kb ; attn = tril(qh kh^T)
  scan:       u = u0 - w @ S ; out = qh @ S + attn @ u ; S += kh^T u

Implementation notes:
- float32r matmuls (TF32-like, ~1.5e-4 rel) with moving operands >=256 wide
  so the full f32r rate engages; narrow rhs are read twice via a 0-stride AP.
- Chunks are processed in PAIRS sharing wide [128, 2, 128] SBUF tiles; the
  two per-chunk matmul results land in a 3-slot PSUM tile (slots 0 and 2
  real) and drain with a single strided copy, halving copy instructions.
"""
import numpy as np

import concourse.bass as bass
import concourse.mybir as mybir
import concourse.tile as tile
from concourse import bacc
from concourse.bass_utils import run_bass_kernel_spmd
from concourse.masks import make_identity, make_lower_triangular, make_upper_triangular

B, H, L, D = 4, 8, 4096, 128
C = 128
NT = L // C
NSEQ = (B * H) // 8   # sequences per core
FP = mybir.dt.float32
FR = mybir.dt.float32r
BF = mybir.dt.bfloat16
EPS = 1e-6
AF = mybir.ActivationFunctionType
ALU = mybir.AluOpType


def dbl(ap):
    """Read a [P, N] AP twice along free dim -> [P, 2, N] so matmul N>=256."""
    return bass.AP(tensor=ap.tensor, offset=ap.offset, ap=[ap.ap[0], [0, 2], ap.ap[1]])


def sl02(ps3):
    """Slots 0 and 2 of a [P, 3, N] psum tile as a [P, 2, N] strided view."""
    ap = ps3.ap
    return bass.AP(tensor=ps3.tensor, offset=ps3.offset,
                   ap=[ap[0], [ap[1][0] * 2, 2], ap[2]])


def _emit_pair(nc, work, pp, pscan, cst, S, bT, dram, s, pr):
    """Emit pass A for chunks (2*pr, 2*pr+1) with wide tiles, then both scans."""
    q_d, k_d, v_d, o_d = dram["q"], dram["k"], dram["v"], dram["out"]
    identR = cst["identR"]
    rows = slice(pr * 2 * C, (pr + 1) * 2 * C)
    rr = lambda ap: ap.rearrange("(two c) d -> c two d", two=2)

    # ---- loads (one DMA per tensor per pair) ----
    qt = work.tile([C, 2, D], FP, tag="q", name="qt")
    kt = work.tile([C, 2, D], FP, tag="k", name="kt")
    vt = work.tile([C, 2, D], FP, tag="v", name="vt")
    nc.sync.dma_start(out=qt, in_=rr(q_d[s, rows, :]))
    nc.sync.dma_start(out=kt, in_=rr(k_d[s, rows, :]))
    nc.sync.dma_start(out=vt, in_=rr(v_d[s, rows, :]))

    # ---- l2 norms + beta scaling ----
    qss = work.tile([C, 2], FP, tag="qss", name="qss")
    kss = work.tile([C, 2], FP, tag="kss", name="kss")
    for j in range(2):
        scr = work.tile([C, D], FP, tag="scr", name="scr")
        scr2 = work.tile([C, D], FP, tag="scr2", name="scr2")
        nc.scalar.activation(out=scr, in_=qt[:, j, :], func=AF.Square, accum_out=qss[:, j:j + 1])
        nc.scalar.activation(out=scr2, in_=kt[:, j, :], func=AF.Square, accum_out=kss[:, j:j + 1])
    nc.scalar.activation(out=qss, in_=qss, func=AF.Sqrt, bias=cst["epsT"][:, 0:1], scale=1.0)
    nc.scalar.activation(out=kss, in_=kss, func=AF.Sqrt, bias=cst["epsT"][:, 0:1], scale=1.0)
    nc.vector.reciprocal(out=qss, in_=qss)
    nc.vector.reciprocal(out=kss, in_=kss)
    qh = work.tile([C, 2, D], FR, tag="qh", name="qh")
    kh = work.tile([C, 2, D], FR, tag="kh", name="kh")
    vb = work.tile([C, 2, D], FR, tag="vb", name="vb")
    kb = work.tile([C, 2, D], FR, tag="kb", name="kb")
    for j in range(2):
        bcol = bT[s][:, 2 * pr + j:2 * pr + j + 1]
        nc.gpsimd.tensor_scalar_mul(qh[:, j, :], qt[:, j, :], qss[:, j:j + 1])
        nc.gpsimd.tensor_scalar_mul(kh[:, j, :], kt[:, j, :], kss[:, j:j + 1])
        nc.gpsimd.tensor_scalar_mul(vb[:, j, :], vt[:, j, :], bcol)
        nc.gpsimd.tensor_scalar(out=kb[:, j, :], in0=kt[:, j, :], scalar1=kss[:, j:j + 1],
                                scalar2=bcol, op0=ALU.mult, op1=ALU.mult)

    # ---- transposes: qT, kT, kbTn wide (per-chunk PE transpose, one copy) ----
    qT_ps = pp.tile([D, 2, C], FR, tag="pa", name="qT_ps")
    nc.tensor.matmul(qT_ps[:, 0, :], qh[:, 0, :], identR, is_transpose=True)
    nc.tensor.matmul(qT_ps[:, 1, :], qh[:, 1, :], identR, is_transpose=True)
    qT = work.tile([D, 2, C], FR, tag="qT", name="qT")
    nc.vector.tensor_copy(qT, qT_ps)
    kT_ps = pp.tile([D, 2, C], FR, tag="pa", name="kT_ps")
    nc.tensor.matmul(kT_ps[:, 0, :], kh[:, 0, :], identR, is_transpose=True)
    nc.tensor.matmul(kT_ps[:, 1, :], kh[:, 1, :], identR, is_transpose=True)
    kT = work.tile([D, 2, C], FR, tag="kT", name="kT")
    nc.vector.tensor_copy(kT, kT_ps)
    kbT_ps = pp.tile([D, 2, C], FR, tag="pa", name="kbT_ps")
    nc.tensor.matmul(kbT_ps[:, 0, :], kb[:, 0, :], identR, is_transpose=True)
    nc.tensor.matmul(kbT_ps[:, 1, :], kb[:, 1, :], identR, is_transpose=True)
    kbTn = work.tile([D, 2, C], FR, tag="kbTn", name="kbTn")
    nc.scalar.activation(out=kbTn, in_=kbT_ps, func=AF.Copy, scale=-1.0)  # -kb^T

    # ---- -T for both chunks -> P0 (masked); PT0 via PE transpose of P0 ----
    T_ps = pp.tile([C, 3, C], FP, tag="pa", name="T_ps")
    nc.tensor.matmul(T_ps[:, 0:2, :], kbTn[:, 0, :], kT, start=True, stop=True)
    nc.tensor.matmul(T_ps[:, 1:3, :], kbTn[:, 1, :], kT, start=True, stop=True)
    P = work.tile([C, 2, C], BF, tag="Pb0", name="P0")
    nc.vector.tensor_mul(P, sl02(T_ps), cst["mSLw"])      # Mp = -tril(T,-1), both chunks
    PT_ps = pp.tile([C, 2, C], BF, tag="pa", name="PT_ps")
    nc.tensor.matmul(PT_ps[:, 0, :], P[:, 0, :], cst["identB"], is_transpose=True)
    nc.tensor.matmul(PT_ps[:, 1, :], P[:, 1, :], cst["identB"], is_transpose=True)
    PT = work.tile([C, 2, C], BF, tag="PTb0", name="PT0")
    nc.scalar.copy(PT, PT_ps)
    R = work.tile([C, 2, C], BF, tag="R", name="R0")
    nc.gpsimd.tensor_add(R, PT, cst["identBw"])           # R = I + PT0

    # ---- Neumann chain: invT = prod_j (I + PT_j), exponents 1..64 ----
    identB = cst["identB"]
    for j in range(6):
        Pn_ps = pp.tile([C, 2, C], FP, tag="pa", name="Pn_ps")
        nc.tensor.matmul(Pn_ps[:, 0, :], PT[:, 0, :], P[:, 0, :], start=True, stop=True)
        nc.tensor.matmul(Pn_ps[:, 1, :], PT[:, 1, :], P[:, 1, :], start=True, stop=True)
        Pn = work.tile([C, 2, C], BF, tag=f"Pb{j+1}", name=f"Pb{j+1}")
        if j % 2 == 0:
            nc.vector.tensor_copy(Pn, Pn_ps)
        else:
            nc.scalar.copy(Pn, Pn_ps)
        if j < 5:
            PTn_ps = pp.tile([C, 2, C], FP, tag="pa", name="PTn_ps")
            nc.tensor.matmul(PTn_ps[:, 0, :], P[:, 0, :], PT[:, 0, :], start=True, stop=True)
            nc.tensor.matmul(PTn_ps[:, 1, :], P[:, 1, :], PT[:, 1, :], start=True, stop=True)
            PTn = work.tile([C, 2, C], BF, tag=f"PTb{j+1}", name=f"PTb{j+1}")
            if j % 2 == 0:
                nc.vector.tensor_copy(PTn, PTn_ps)
            else:
                nc.scalar.copy(PTn, PTn_ps)
            PT = PTn
        # R <- (I + PT_{j+1}) R  via per-chunk (P^T R + I R) psum accumulation
        Rn_ps = pp.tile([C, 2, C], FP, tag="pa", name="Rn_ps")
        nc.tensor.matmul(Rn_ps[:, 0, :], Pn[:, 0, :], R[:, 0, :], start=True, stop=False)
        nc.tensor.matmul(Rn_ps[:, 0, :], identB, R[:, 0, :], start=False, stop=True)
        nc.tensor.matmul(Rn_ps[:, 1, :], Pn[:, 1, :], R[:, 1, :], start=True, stop=False)
        nc.tensor.matmul(Rn_ps[:, 1, :], identB, R[:, 1, :], start=False, stop=True)
        P = Pn
        if j < 5:
            R = work.tile([C, 2, C], BF, tag="R", name=f"R{j+1}")
        else:
            R = work.tile([C, 2, C], FR, tag="Rf", name="Rf")  # final inverse in f32r
        if j % 2 == 0:
            nc.scalar.copy(R, Rn_ps)
        else:
            nc.vector.tensor_copy(R, Rn_ps)
    invT = R  # [e, 2, c] = inv^T per chunk (f32r)

    # ---- u0, w^T (negated), attn^T ----
    u0_ps = pp.tile([C, 3, D], FP, tag="pa", name="u0_ps")
    nc.tensor.matmul(u0_ps[:, 0:2, :], invT[:, 0, :], vb, start=True, stop=True)
    nc.tensor.matmul(u0_ps[:, 1:3, :], invT[:, 1, :], vb, start=True, stop=True)
    u0 = work.tile([C, 2, D], FP, tag="u0", name="u0")
    nc.vector.tensor_copy(u0, sl02(u0_ps))
    w_ps = pp.tile([D, 3, C], FP, tag="pa", name="w_ps")
    nc.tensor.matmul(w_ps[:, 0:2, :], kb[:, 0, :], invT, start=True, stop=True)
    nc.tensor.matmul(w_ps[:, 1:3, :], kb[:, 1, :], invT, start=True, stop=True)
    wTn = work.tile([D, 2, C], FR, tag="wTn", name="wTn")
    nc.scalar.activation(out=wTn, in_=sl02(w_ps), func=AF.Copy, scale=-1.0)
    a_ps = pp.tile([C, 3, C], FP, tag="pa", name="a_ps")
    nc.tensor.matmul(a_ps[:, 0:2, :], kT[:, 0, :], qT, start=True, stop=True)
    nc.tensor.matmul(a_ps[:, 1:3, :], kT[:, 1, :], qT, start=True, stop=True)
    attnT = work.tile([C, 2, C], FR, tag="attnT", name="attnT")
    nc.vector.tensor_mul(attnT, sl02(a_ps), cst["mUIw"])  # keep c2<=c1

    # ---- scan steps (sequential in chunk index per sequence) ----
    St = S[s]
    out_sb = work.tile([C, 2, D], FP, tag="outsb", name="out_sb")
    for j in range(2):
        u_ps = pscan.tile([C, 2, D], FP, tag="u", name="u_ps", bufs=1)
        nc.tensor.matmul(u_ps, wTn[:, j, :], dbl(St), start=True, stop=True)    # -w @ S
        u = work.tile([C, D], FR, tag="u", name="u")
        nc.vector.tensor_add(u, u_ps[:, 0, :], u0[:, j, :])                     # u = u0 - w S
        out_ps = pscan.tile([C, 2, D], FP, tag="out", name="out_ps", bufs=1)
        nc.tensor.matmul(out_ps, qT[:, j, :], dbl(St), start=True, stop=False)  # qh @ S
        nc.tensor.matmul(out_ps, attnT[:, j, :], dbl(u), start=False, stop=True)
        sd_ps = pscan.tile([D, 2, D], FP, tag="sd", name="sd_ps", bufs=1)
        nc.tensor.matmul(sd_ps, kh[:, j, :], dbl(u), start=True, stop=True)     # kh^T u
        nc.vector.tensor_add(St, St, sd_ps[:, 0, :])                            # S += kh^T u
        nc.scalar.copy(out_sb[:, j, :], out_ps[:, 0, :])
    nc.sync.dma_start(out=rr(o_d[s, rows, :]), in_=out_sb)


def build_nc(nseq=NSEQ, nt=NT, repeat=1):
    assert nt % 2 == 0
    ll = nt * C
    nc = bacc.Bacc(None, target_bir_lowering=False)
    dram = {
        "q": nc.dram_tensor("q", [nseq, ll, D], FP, kind="ExternalInput"),
        "k": nc.dram_tensor("k", [nseq, ll, D], FP, kind="ExternalInput"),
        "v": nc.dram_tensor("v", [nseq, ll, D], FP, kind="ExternalInput"),
        "beta": nc.dram_tensor("beta", [nseq, ll], FP, kind="ExternalInput"),
        "out": nc.dram_tensor("out", [nseq, ll, D], FP, kind="ExternalOutput"),
    }
    with tile.TileContext(nc) as tc:
        with (
            tc.tile_pool(name="consts", bufs=1) as consts,
            tc.tile_pool(name="persist", bufs=1) as persist,
            tc.tile_pool(name="work", bufs=6) as work,
            tc.tile_pool(name="pp", bufs=5, space="PSUM") as pp,
            tc.tile_pool(name="pscan", bufs=2, space="PSUM") as pscan,
        ):
            ident = consts.tile([128, 128], FP, tag="ident", name="ident")
            identR = consts.tile([128, 128], FR, tag="identR", name="identR")
            identRw = consts.tile([128, 2, 128], FR, tag="identRw", name="identRw")
            identB = consts.tile([128, 128], BF, tag="identB", name="identB")
            identBw = consts.tile([128, 2, 128], BF, tag="identBw", name="identBw")
            mSLw = consts.tile([128, 2, 128], FP, tag="mSLw", name="mSLw")
            mUIw = consts.tile([128, 2, 128], FP, tag="mUIw", name="mUIw")
            epsT = consts.tile([128, 1], FP, tag="epsT", name="epsT")
            zeros = consts.tile([128, 128], FP, tag="zeros", name="zeros")
            make_identity(nc, ident)
            nc.vector.tensor_copy(identR, ident)
            nc.vector.tensor_copy(identRw[:, 0, :], ident)
            nc.vector.tensor_copy(identRw[:, 1, :], ident)
            nc.vector.tensor_copy(identB, ident)
            nc.vector.tensor_copy(identBw[:, 0, :], ident)
            nc.vector.tensor_copy(identBw[:, 1, :], ident)
            for j in range(2):
                make_lower_triangular(nc, mSLw[:, j, :], val=1.0, diag=False)
                make_upper_triangular(nc, mUIw[:, j, :], val=1.0, diag=True)
            nc.gpsimd.memset(epsT, EPS)
            nc.gpsimd.memset(zeros, 0.0)
            cst = dict(ident=ident, identR=identR, identRw=identRw,
                       identB=identB, identBw=identBw,
                       mSLw=mSLw, mUIw=mUIw, epsT=epsT)

            S, bT = [], []
            for s in range(nseq):
                St = persist.tile([D, D], FR, tag=f"S{s}", name=f"S{s}")
                nc.vector.tensor_copy(St, zeros)
                S.append(St)
                bseq = persist.tile([nt, C], FP, tag=f"bseq{s}", name=f"bseq{s}")
                nc.sync.dma_start(out=bseq, in_=dram["beta"][s].rearrange("(n c) -> n c", c=C))
                bt_ps = pp.tile([C, nt], FP, tag="pa", name=f"btps{s}")
                nc.tensor.transpose(bt_ps, bseq, ident[:nt, :nt])
                btile = persist.tile([C, nt], FP, tag=f"bT{s}", name=f"bT{s}")
                nc.vector.tensor_copy(btile, bt_ps)
                bT.append(btile)

            for rep in range(repeat):
                if rep > 0:
                    for s in range(nseq):
                        nc.vector.tensor_copy(S[s], zeros)
                for pr in range(nt // 2):
                    for s in range(nseq):
                        _emit_pair(nc, work, pp, pscan, cst, S, bT, dram, s, pr)
    nc.compile()
    return nc


_NC_CACHE = None


def _build_in_maps(inputs):
    q = np.ascontiguousarray(np.asarray(inputs["q"], dtype=np.float32))
    k = np.ascontiguousarray(np.asarray(inputs["k"], dtype=np.float32))
    v = np.ascontiguousarray(np.asarray(inputs["v"], dtype=np.float32))
    beta = np.ascontiguousarray(np.asarray(inputs["beta"], dtype=np.float32))
    qf = q.reshape(B * H, L, D)
    kf = k.reshape(B * H, L, D)
    vf = v.reshape(B * H, L, D)
    bf = beta.reshape(B * H, L)
    in_maps = []
    for core in range(8):
        sl = slice(core * NSEQ, (core + 1) * NSEQ)
        in_maps.append({
            "q": np.ascontiguousarray(qf[sl]),
            "k": np.ascontiguousarray(kf[sl]),
            "v": np.ascontiguousarray(vf[sl]),
            "beta": np.ascontiguousarray(bf[sl]),
        })
    return in_maps


def kernel(q, k, v, beta):
    global _NC_CACHE
    if _NC_CACHE is None:
        _NC_CACHE = build_nc()
    nc = _NC_CACHE
    in_maps = _build_in_maps({"q": q, "k": k, "v": v, "beta": beta})
    res = run_bass_kernel_spmd(nc, in_maps, core_ids=list(range(8)))
    out = np.empty((B * H, L, D), dtype=np.float32)
    for core in range(8):
        out[core * NSEQ:(core + 1) * NSEQ] = res.results[core]["out"]
    return out.reshape(B, H, L, D)


# revision 15
# speedup vs baseline: 704.7277x; 9.6078x over previous
"""DeltaNet chunked delta-rule kernel for Trainium2 (Bass/Tile), 8-core SPMD.

Full inputs: q,k,v [4,8,4096,128] fp32, beta [4,8,4096] fp32.
Sharding: 32 (b,h) pairs -> 4 per core across 8 cores (state S is per (b,h)).

Algorithm (mathematically identical to the CHUNK=32 reference for any chunk
size; we use C=128):
  qh = l2norm(q), kh = l2norm(k), vb = v*beta, kb = kh*beta
  per chunk:  T = kb @ kh^T;  M = I + tril(T,-1);  inv = M^-1
              (computed exactly via the nilpotent Neumann product
               inv = prod_j (I + Mp^(2^j)), Mp = -tril(T,-1))
              u0 = inv @ vb ; w = inv @ kb ; attn = tril(qh kh^T)
  scan:       u = u0 - w @ S ; out = qh @ S + attn @ u ; S += kh^T u

Implementation notes:
- float32r matmuls (TF32-like, ~1.5e-4 rel) with moving operands >=256 wide
  so the full f32r rate engages; narrow rhs are read twice via a 0-stride AP.
- Chunks are processed in PAIRS sharing wide [128, 2, 128] SBUF tiles; the
  two per-chunk matmul results land in a 3-slot PSUM tile (slots 0 and 2
  real) and drain with a single strided copy, halving copy instructions.
"""
import numpy as np

import concourse.bass as bass
import concourse.mybir as mybir
import concourse.tile as tile
from concourse import bacc
from concourse.bass_utils import run_bass_kernel_spmd
from concourse.masks import make_identity, make_lower_triangular, make_upper_triangular

B, H, L, D = 4, 8, 4096, 128
C = 128
NT = L // C
NSEQ = (B * H) // 8   # sequences per core
FP = mybir.dt.float32
FR = mybir.dt.float32r
BF = mybir.dt.bfloat16
EPS = 1e-6
AF = mybir.ActivationFunctionType
ALU = mybir.AluOpType


def dbl(ap):
    """Read a [P, N] AP twice along free dim -> [P, 2, N] so matmul N>=256."""
    return bass.AP(tensor=ap.tensor, offset=ap.offset, ap=[ap.ap[0], [0, 2], ap.ap[1]])


def sl02(ps3):
    """Slots 0 and 2 of a [P, 3, N] psum tile as a [P, 2, N] strided view."""
    ap = ps3.ap
    return bass.AP(tensor=ps3.tensor, offset=ps3.offset,
                   ap=[ap[0], [ap[1][0] * 2, 2], ap[2]])


def _emit_pair(nc, work, pp, pscan, cst, S, bT, dram, s, pr, dma_only=False):
    """Emit pass A for chunks (2*pr, 2*pr+1) with wide tiles, then both scans."""
    q_d, k_d, v_d, o_d = dram["q"], dram["k"], dram["v"], dram["out"]
    identR = cst["identR"]
    rows = slice(pr * 2 * C, (pr + 1) * 2 * C)
    rr = lambda ap: ap.rearrange("(two c) d -> c two d", two=2)

    # ---- loads (one DMA per tensor per pair) ----
    qt = work.tile([C, 2, D], FP, tag="q", name="qt")
    kt = work.tile([C, 2, D], FP, tag="k", name="kt")
    vt = work.tile([C, 2, D], FP, tag="v", name="vt")
    nc.sync.dma_start(out=qt, in_=rr(q_d[s, rows, :]))
    nc.sync.dma_start(out=kt, in_=rr(k_d[s, rows, :]))
    nc.sync.dma_start(out=vt, in_=rr(v_d[s, rows, :]))
    if dma_only:
        out_sb = work.tile([C, 2, D], FP, tag="outsb", name="out_sb")
        nc.vector.tensor_add(out_sb, qt, kt)
        nc.sync.dma_start(out=rr(o_d[s, rows, :]), in_=out_sb)
        return

    # ---- l2 norms + beta scaling ----
    qss = work.tile([C, 2], FP, tag="qss", name="qss")
    kss = work.tile([C, 2], FP, tag="kss", name="kss")
    for j in range(2):
        scr = work.tile([C, D], FP, tag="scr", name="scr")
        scr2 = work.tile([C, D], FP, tag="scr2", name="scr2")
        nc.scalar.activation(out=scr, in_=qt[:, j, :], func=AF.Square, accum_out=qss[:, j:j + 1])
        nc.scalar.activation(out=scr2, in_=kt[:, j, :], func=AF.Square, accum_out=kss[:, j:j + 1])
    nc.scalar.activation(out=qss, in_=qss, func=AF.Sqrt, bias=cst["epsT"][:, 0:1], scale=1.0)
    nc.scalar.activation(out=kss, in_=kss, func=AF.Sqrt, bias=cst["epsT"][:, 0:1], scale=1.0)
    nc.vector.reciprocal(out=qss, in_=qss)
    nc.vector.reciprocal(out=kss, in_=kss)
    qh = work.tile([C, 2, D], FR, tag="qh", name="qh")
    kh = work.tile([C, 2, D], FR, tag="kh", name="kh")
    vb = work.tile([C, 2, D], FR, tag="vb", name="vb")
    kb = work.tile([C, 2, D], FR, tag="kb", name="kb")
    for j in range(2):
        bcol = bT[s][:, 2 * pr + j:2 * pr + j + 1]
        nc.gpsimd.tensor_scalar_mul(qh[:, j, :], qt[:, j, :], qss[:, j:j + 1])
        nc.gpsimd.tensor_scalar_mul(kh[:, j, :], kt[:, j, :], kss[:, j:j + 1])
        nc.gpsimd.tensor_scalar_mul(vb[:, j, :], vt[:, j, :], bcol)
        nc.gpsimd.tensor_scalar(out=kb[:, j, :], in0=kt[:, j, :], scalar1=kss[:, j:j + 1],
                                scalar2=bcol, op0=ALU.mult, op1=ALU.mult)

    # ---- transposes: qT, kT, kbTn wide (per-chunk PE transpose, one copy) ----
    qT_ps = pp.tile([D, 2, C], FR, tag="pa", name="qT_ps")
    nc.tensor.matmul(qT_ps[:, 0, :], qh[:, 0, :], identR, is_transpose=True)
    nc.tensor.matmul(qT_ps[:, 1, :], qh[:, 1, :], identR, is_transpose=True)
    qT = work.tile([D, 2, C], FR, tag="qT", name="qT")
    nc.vector.tensor_copy(qT, qT_ps)
    kT_ps = pp.tile([D, 2, C], FR, tag="pa", name="kT_ps")
    nc.tensor.matmul(kT_ps[:, 0, :], kh[:, 0, :], identR, is_transpose=True)
    nc.tensor.matmul(kT_ps[:, 1, :], kh[:, 1, :], identR, is_transpose=True)
    kT = work.tile([D, 2, C], FR, tag="kT", name="kT")
    nc.vector.tensor_copy(kT, kT_ps)
    kbT_ps = pp.tile([D, 2, C], FR, tag="pa", name="kbT_ps")
    nc.tensor.matmul(kbT_ps[:, 0, :], kb[:, 0, :], identR, is_transpose=True)
    nc.tensor.matmul(kbT_ps[:, 1, :], kb[:, 1, :], identR, is_transpose=True)
    kbTn = work.tile([D, 2, C], FR, tag="kbTn", name="kbTn")
    nc.scalar.activation(out=kbTn, in_=kbT_ps, func=AF.Copy, scale=-1.0)  # -kb^T

    # ---- -T for both chunks -> P0 (masked); PT0 via PE transpose of P0 ----
    T_ps = pp.tile([C, 3, C], FP, tag="pa", name="T_ps")
    nc.tensor.matmul(T_ps[:, 0:2, :], kbTn[:, 0, :], kT, start=True, stop=True)
    nc.tensor.matmul(T_ps[:, 1:3, :], kbTn[:, 1, :], kT, start=True, stop=True)
    P = work.tile([C, 2, C], BF, tag="Pb0", name="P0")
    nc.vector.tensor_mul(P, sl02(T_ps), cst["mSLw"])      # Mp = -tril(T,-1), both chunks
    PT_ps = pp.tile([C, 2, C], BF, tag="pa", name="PT_ps")
    nc.tensor.matmul(PT_ps[:, 0, :], P[:, 0, :], cst["identB"], is_transpose=True)
    nc.tensor.matmul(PT_ps[:, 1, :], P[:, 1, :], cst["identB"], is_transpose=True)
    PT = work.tile([C, 2, C], BF, tag="PTb0", name="PT0")
    nc.scalar.copy(PT, PT_ps)
    R = work.tile([C, 2, C], BF, tag="R", name="R0")
    nc.gpsimd.tensor_add(R, PT, cst["identBw"])           # R = I + PT0

    # ---- Neumann chain: invT = prod_j (I + PT_j), exponents 1..64 ----
    identB = cst["identB"]
    for j in range(6):
        Pn_ps = pp.tile([C, 2, C], FP, tag="pa", name="Pn_ps")
        nc.tensor.matmul(Pn_ps[:, 0, :], PT[:, 0, :], P[:, 0, :], start=True, stop=True)
        nc.tensor.matmul(Pn_ps[:, 1, :], PT[:, 1, :], P[:, 1, :], start=True, stop=True)
        Pn = work.tile([C, 2, C], BF, tag=f"Pb{j+1}", name=f"Pb{j+1}")
        if j % 2 == 0:
            nc.vector.tensor_copy(Pn, Pn_ps)
        else:
            nc.scalar.copy(Pn, Pn_ps)
        if j < 5:
            PTn_ps = pp.tile([C, 2, C], FP, tag="pa", name="PTn_ps")
            nc.tensor.matmul(PTn_ps[:, 0, :], P[:, 0, :], PT[:, 0, :], start=True, stop=True)
            nc.tensor.matmul(PTn_ps[:, 1, :], P[:, 1, :], PT[:, 1, :], start=True, stop=True)
            PTn = work.tile([C, 2, C], BF, tag=f"PTb{j+1}", name=f"PTb{j+1}")
            if j % 2 == 0:
                nc.vector.tensor_copy(PTn, PTn_ps)
            else:
                nc.scalar.copy(PTn, PTn_ps)
            PT = PTn
        # R <- (I + PT_{j+1}) R  via per-chunk (P^T R + I R) psum accumulation
        Rn_ps = pp.tile([C, 2, C], FP, tag="pa", name="Rn_ps")
        nc.tensor.matmul(Rn_ps[:, 0, :], Pn[:, 0, :], R[:, 0, :], start=True, stop=False)
        nc.tensor.matmul(Rn_ps[:, 0, :], identB, R[:, 0, :], start=False, stop=True)
        nc.tensor.matmul(Rn_ps[:, 1, :], Pn[:, 1, :], R[:, 1, :], start=True, stop=False)
        nc.tensor.matmul(Rn_ps[:, 1, :], identB, R[:, 1, :], start=False, stop=True)
        P = Pn
        if j < 5:
            R = work.tile([C, 2, C], BF, tag="R", name=f"R{j+1}")
        else:
            R = work.tile([C, 2, C], FR, tag="Rf", name="Rf")  # final inverse in f32r
        if j % 2 == 0:
            nc.scalar.copy(R, Rn_ps)
        else:
            nc.vector.tensor_copy(R, Rn_ps)
    invT = R  # [e, 2, c] = inv^T per chunk (f32r)

    # ---- u0, w^T (negated), attn^T ----
    u0_ps = pp.tile([C, 3, D], FP, tag="pa", name="u0_ps")
    nc.tensor.matmul(u0_ps[:, 0:2, :], invT[:, 0, :], vb, start=True, stop=True)
    nc.tensor.matmul(u0_ps[:, 1:3, :], invT[:, 1, :], vb, start=True, stop=True)
    u0 = work.tile([C, 2, D], FP, tag="u0", name="u0")
    nc.vector.tensor_copy(u0, sl02(u0_ps))
    w_ps = pp.tile([D, 3, C], FP, tag="pa", name="w_ps")
    nc.tensor.matmul(w_ps[:, 0:2, :], kb[:, 0, :], invT, start=True, stop=True)
    nc.tensor.matmul(w_ps[:, 1:3, :], kb[:, 1, :], invT, start=True, stop=True)
    wTn = work.tile([D, 2, C], FR, tag="wTn", name="wTn")
    nc.scalar.activation(out=wTn, in_=sl02(w_ps), func=AF.Copy, scale=-1.0)
    a_ps = pp.tile([C, 3, C], FP, tag="pa", name="a_ps")
    nc.tensor.matmul(a_ps[:, 0:2, :], kT[:, 0, :], qT, start=True, stop=True)
    nc.tensor.matmul(a_ps[:, 1:3, :], kT[:, 1, :], qT, start=True, stop=True)
    attnT = work.tile([C, 2, C], FR, tag="attnT", name="attnT")
    nc.vector.tensor_mul(attnT, sl02(a_ps), cst["mUIw"])  # keep c2<=c1

    # ---- scan steps (sequential in chunk index per sequence) ----
    St = S[s]
    out_sb = work.tile([C, 2, D], FP, tag="outsb", name="out_sb")
    for j in range(2):
        u_ps = pscan.tile([C, 2, D], FP, tag="u", name="u_ps", bufs=1)
        nc.tensor.matmul(u_ps, wTn[:, j, :], dbl(St), start=True, stop=True)    # -w @ S
        u = work.tile([C, D], FR, tag="u", name="u")
        nc.vector.tensor_add(u, u_ps[:, 0, :], u0[:, j, :])                     # u = u0 - w S
        out_ps = pscan.tile([C, 2, D], FP, tag="out", name="out_ps", bufs=1)
        nc.tensor.matmul(out_ps, qT[:, j, :], dbl(St), start=True, stop=False)  # qh @ S
        nc.tensor.matmul(out_ps, attnT[:, j, :], dbl(u), start=False, stop=True)
        sd_ps = pscan.tile([D, 2, D], FP, tag="sd", name="sd_ps", bufs=1)
        nc.tensor.matmul(sd_ps, kh[:, j, :], dbl(u), start=True, stop=True)     # kh^T u
        nc.vector.tensor_add(St, St, sd_ps[:, 0, :])                            # S += kh^T u
        nc.scalar.copy(out_sb[:, j, :], out_ps[:, 0, :])
    nc.sync.dma_start(out=rr(o_d[s, rows, :]), in_=out_sb)


def build_nc(nseq=NSEQ, nt=NT, repeat=1, dma_only=False):
    assert nt % 2 == 0
    ll = nt * C
    nc = bacc.Bacc(None, target_bir_lowering=False)
    dram = {
        "q": nc.dram_tensor("q", [nseq, ll, D], FP, kind="ExternalInput"),
        "k": nc.dram_tensor("k", [nseq, ll, D], FP, kind="ExternalInput"),
        "v": nc.dram_tensor("v", [nseq, ll, D], FP, kind="ExternalInput"),
        "beta": nc.dram_tensor("beta", [nseq, ll], FP, kind="ExternalInput"),
        "out": nc.dram_tensor("out", [nseq, ll, D], FP, kind="ExternalOutput"),
    }
    with tile.TileContext(nc) as tc:
        with (
            tc.tile_pool(name="consts", bufs=1) as consts,
            tc.tile_pool(name="persist", bufs=1) as persist,
            tc.tile_pool(name="work", bufs=6) as work,
            tc.tile_pool(name="pp", bufs=5, space="PSUM") as pp,
            tc.tile_pool(name="pscan", bufs=2, space="PSUM") as pscan,
        ):
            ident = consts.tile([128, 128], FP, tag="ident", name="ident")
            identR = consts.tile([128, 128], FR, tag="identR", name="identR")
            identRw = consts.tile([128, 2, 128], FR, tag="identRw", name="identRw")
            identB = consts.tile([128, 128], BF, tag="identB", name="identB")
            identBw = consts.tile([128, 2, 128], BF, tag="identBw", name="identBw")
            mSLw = consts.tile([128, 2, 128], FP, tag="mSLw", name="mSLw")
            mUIw = consts.tile([128, 2, 128], FP, tag="mUIw", name="mUIw")
            epsT = consts.tile([128, 1], FP, tag="epsT", name="epsT")
            zeros = consts.tile([128, 128], FP, tag="zeros", name="zeros")
            make_identity(nc, ident)
            nc.vector.tensor_copy(identR, ident)
            nc.vector.tensor_copy(identRw[:, 0, :], ident)
            nc.vector.tensor_copy(identRw[:, 1, :], ident)
            nc.vector.tensor_copy(identB, ident)
            nc.vector.tensor_copy(identBw[:, 0, :], ident)
            nc.vector.tensor_copy(identBw[:, 1, :], ident)
            for j in range(2):
                make_lower_triangular(nc, mSLw[:, j, :], val=1.0, diag=False)
                make_upper_triangular(nc, mUIw[:, j, :], val=1.0, diag=True)
            nc.gpsimd.memset(epsT, EPS)
            nc.gpsimd.memset(zeros, 0.0)
            cst = dict(ident=ident, identR=identR, identRw=identRw,
                       identB=identB, identBw=identBw,
                       mSLw=mSLw, mUIw=mUIw, epsT=epsT)

            S, bT = [], []
            for s in range(nseq):
                St = persist.tile([D, D], FR, tag=f"S{s}", name=f"S{s}")
                nc.vector.tensor_copy(St, zeros)
                S.append(St)
                bseq = persist.tile([nt, C], FP, tag=f"bseq{s}", name=f"bseq{s}")
                nc.sync.dma_start(out=bseq, in_=dram["beta"][s].rearrange("(n c) -> n c", c=C))
                bt_ps = pp.tile([C, nt], FP, tag="pa", name=f"btps{s}")
                nc.tensor.transpose(bt_ps, bseq, ident[:nt, :nt])
                btile = persist.tile([C, nt], FP, tag=f"bT{s}", name=f"bT{s}")
                nc.vector.tensor_copy(btile, bt_ps)
                bT.append(btile)

            for rep in range(repeat):
                if rep > 0:
                    for s in range(nseq):
                        nc.vector.tensor_copy(S[s], zeros)
                for pr in range(nt // 2):
                    for s in range(nseq):
                        _emit_pair(nc, work, pp, pscan, cst, S, bT, dram, s, pr, dma_only=dma_only)
    nc.compile()
    return nc


_NC_CACHE = None


def _build_in_maps(inputs):
    q = np.ascontiguousarray(np.asarray(inputs["q"], dtype=np.float32))
    k = np.ascontiguousarray(np.asarray(inputs["k"], dtype=np.float32))
    v = np.ascontiguousarray(np.asarray(inputs["v"], dtype=np.float32))
    beta = np.ascontiguousarray(np.asarray(inputs["beta"], dtype=np.float32))
    qf = q.reshape(B * H, L, D)
    kf = k.reshape(B * H, L, D)
    vf = v.reshape(B * H, L, D)
    bf = beta.reshape(B * H, L)
    in_maps = []
    for core in range(8):
        sl = slice(core * NSEQ, (core + 1) * NSEQ)
        in_maps.append({
            "q": np.ascontiguousarray(qf[sl]),
            "k": np.ascontiguousarray(kf[sl]),
            "v": np.ascontiguousarray(vf[sl]),
            "beta": np.ascontiguousarray(bf[sl]),
        })
    return in_maps


def kernel(q, k, v, beta):
    global _NC_CACHE
    if _NC_CACHE is None:
        _NC_CACHE = build_nc()
    nc = _NC_CACHE
    in_maps = _build_in_maps({"q": q, "k": k, "v": v, "beta": beta})
    res = run_bass_kernel_spmd(nc, in_maps, core_ids=list(range(8)))
    out = np.empty((B * H, L, D), dtype=np.float32)
    for core in range(8):
        out[core * NSEQ:(core + 1) * NSEQ] = res.results[core]["out"]
    return out.reshape(B, H, L, D)
